# revision 1
# baseline (speedup 1.0000x reference)
"""Jamba sparse-MoE block on 8 Trainium2 NeuronCores (expert-parallel).

Strategy
--------
- Routing (router matmul + softmax + top-2) is computed with jax on the host
  using the exact op sequence of the reference so expert selection matches
  bit-for-bit (one token in the dataset has a top2/top3 probability gap of
  ~5e-7; any rounding difference there would flip its expert assignment).
- Tokens are dispatched (gathered) per expert on the host; core e runs the
  dense gate/up/silu/mul/down FFN of expert e over its ~2k assigned tokens.
  This is the "all-to-all dispatch by top_k_index + expert-parallel weights"
  sharding, with the dispatch done at input-sharding time.
- Each core's Bass kernel is PE-bound and runs matmuls as float32r (full PE
  rate at N>=256, ~1.5e-4 relative rounding) with fp32 PSUM accumulation:
    phase A: hid = silu(x @ gw.T) * (x @ uw.T)   [F x C], staged to DRAM
    phase B: y = (w_token * (hid.T @ dw.T))      [C x H]
  The down-projection weights are fully cached in SBUF (one half preloaded
  during phase A, the other after the x tile is freed), so hid streams
  through phase B exactly once and the PE stays the bottleneck.
- Outputs are scatter-added back into the full [T, H] buffer on the host
  (each token appears in exactly two experts' outputs).
"""

import math
import numpy as np
from contextlib import ExitStack

B, S, H, F, E, TOP_K = 4, 2048, 1024, 4096, 8, 2
T = B * S
N_CORES = 8
P = 128
HC = H // P  # 8 h-chunks
FB = F // P  # 32 f-blocks


def _token_tiles(C):
    assert C % 256 == 0 and C >= 256
    tiles = [512] * (C // 512)
    if C % 512:
        tiles.append(C % 512)
    return tiles


_PROGRAM_CACHE = {}


def _build_program(C, H_=H, F_=F, act="Silu"):
    """SPMD program for one expert's FFN over C token slots."""
    key = (C, H_, F_, act)
    if key in _PROGRAM_CACHE:
        return _PROGRAM_CACHE[key]
    import concourse.bacc as bacc
    import concourse.mybir as mybir
    import concourse.tile as tile

    HC = H_ // P
    FB = F_ // P
    FBH = (3 * FB) // 4  # dw cache split: big half preloaded during phase A
    NH = max(1, H_ // 512)  # matmul slices along H
    HW_ = H_ // NH
    f32 = mybir.dt.float32
    f32r = mybir.dt.float32r
    AF = mybir.ActivationFunctionType
    NT128 = C // P
    NT256 = C // 256
    tiles = _token_tiles(C)

    nc = bacc.Bacc("TRN2", target_bir_lowering=False, debug=False, num_devices=N_CORES)

    x_d = nc.dram_tensor("x", [P, HC, C], f32r, kind="ExternalInput")
    gw_d = nc.dram_tensor("gw", [FB, P, HC, P], f32r, kind="ExternalInput")
    uw_d = nc.dram_tensor("uw", [FB, P, HC, P], f32r, kind="ExternalInput")
    dw_d = nc.dram_tensor("dw", [P, FB, H_], f32r, kind="ExternalInput")
    wt_d = nc.dram_tensor("wt", [NT128, P], f32, kind="ExternalInput")
    y_d = nc.dram_tensor("y", [NT128, P, H_], f32, kind="ExternalOutput")
    hid_d = nc.dram_tensor("hid", [FB, P, C], f32r)  # internal staging

    with tile.TileContext(nc) as tc:
        with ExitStack() as ctx:
            wtpool = ctx.enter_context(tc.tile_pool(name="wtp", bufs=1))
            dlpool = ctx.enter_context(tc.tile_pool(name="dlp", bufs=1))

            wt_t = wtpool.tile([P, NT128], f32)
            nc.sync.dma_start(wt_t[:], wt_d.ap().rearrange("n p -> p n"))
            # first dw half: preloaded while phase A runs
            dw_lo = dlpool.tile([P, FBH, H_], f32r)
            nc.gpsimd.dma_start(dw_lo[:], dw_d.ap()[:, :FBH, :])

            # ---- Phase A: hid[f, t] = silu(g) * u, staged to DRAM ----
            with ExitStack() as actx:
                psa = actx.enter_context(tc.tile_pool(name="psa", bufs=3, space="PSUM"))
                xpool = actx.enter_context(tc.tile_pool(name="xp", bufs=1))
                gwpool = actx.enter_context(tc.tile_pool(name="gwp", bufs=3))
                uwpool = actx.enter_context(tc.tile_pool(name="uwp", bufs=3))
                sgpool = actx.enter_context(tc.tile_pool(name="sgp", bufs=2))
                hspool = actx.enter_context(tc.tile_pool(name="hsp", bufs=3))

                x_t = xpool.tile([P, HC, C], f32r)
                t0 = 0
                xchunks = [256, 256] + list(tiles[1:]) if tiles[0] == 512 else tiles
                for nt in xchunks:
                    nc.sync.dma_start(
                        x_t[:, :, t0 : t0 + nt], x_d.ap()[:, :, t0 : t0 + nt]
                    )
                    t0 += nt

                for fb in range(FB):
                    gw_t = gwpool.tile([P, HC, P], f32r)
                    nc.sync.dma_start(gw_t[:], gw_d.ap()[fb])
                    uw_t = uwpool.tile([P, HC, P], f32r)
                    nc.sync.dma_start(uw_t[:], uw_d.ap()[fb])
                    t0 = 0
                    for nt in tiles:
                        ps_g = psa.tile([P, 512], f32, name="ps_g")[:, :nt]
                        ps_u = psa.tile([P, 512], f32, name="ps_u")[:, :nt]
                        for hc in range(HC):
                            nc.tensor.matmul(
                                ps_g,
                                gw_t[:, hc, :],
                                x_t[:, hc, t0 : t0 + nt],
                                start=(hc == 0),
                                stop=(hc == HC - 1),
                            )
                        for hc in range(HC):
                            nc.tensor.matmul(
                                ps_u,
                                uw_t[:, hc, :],
                                x_t[:, hc, t0 : t0 + nt],
                                start=(hc == 0),
                                stop=(hc == HC - 1),
                            )
                        sg = sgpool.tile([P, 512], f32, name="sg")[:, :nt]
                        nc.scalar.activation(sg, ps_g, getattr(AF, act))
                        hid_sb = hspool.tile([P, 512], f32r, name="hid_sb")[:, :nt]
                        nc.vector.tensor_mul(hid_sb, sg, ps_u)
                        nc.scalar.dma_start(hid_d.ap()[fb][:, t0 : t0 + nt], hid_sb)
                        t0 += nt

            # ---- Phase B: y[t, :] = w[t] * (hid[:, t].T @ dw.T) ----
            psb = ctx.enter_context(tc.tile_pool(name="psb", bufs=3, space="PSUM"))
            dhpool = ctx.enter_context(tc.tile_pool(name="dhp", bufs=1))
            htpool = ctx.enter_context(tc.tile_pool(name="htp", bufs=2))
            ypool = ctx.enter_context(tc.tile_pool(name="yp", bufs=2))

            dw_hi = dhpool.tile([P, FB - FBH, H_], f32r)
            nc.gpsimd.dma_start(dw_hi[:], dw_d.ap()[:, FBH:, :])

            for tt2 in range(NT256):
                hid_t = htpool.tile([P, FB, 256], f32r, name="hid_t")
                nc.sync.dma_start(
                    hid_t[:], hid_d.ap()[:, :, tt2 * 256 : (tt2 + 1) * 256]
                    .rearrange("b f t -> f b t")
                )
                for sub in range(2):
                    tt = tt2 * 2 + sub
                    ps_y = psb.tile([P, H_], f32, name="ps_y")
                    for fb in range(FB):
                        dwt = dw_lo if fb < FBH else dw_hi
                        fbi = fb if fb < FBH else fb - FBH
                        for nh in range(NH):
                            nc.tensor.matmul(
                                ps_y[:, nh * HW_ : (nh + 1) * HW_],
                                hid_t[:, fb, sub * P : (sub + 1) * P],
                                dwt[:, fbi, nh * HW_ : (nh + 1) * HW_],
                                start=(fb == 0),
                                stop=(fb == FB - 1),
                            )
                    y_sb = ypool.tile([P, H_], f32, name="y_sb")
                    nc.scalar.activation(
                        y_sb[:], ps_y[:], AF.Copy, scale=wt_t[:, tt : tt + 1]
                    )
                    nc.scalar.dma_start(y_d.ap()[tt], y_sb[:])
    nc.compile()
    _PROGRAM_CACHE[key] = nc
    return nc


def _routing(hidden_states, router_w):
    """Replicate the reference's routing ops exactly (same jax ops, default
    platform) so top-2 selection matches bit-for-bit."""
    import jax
    import jax.numpy as jnp

    x = jnp.asarray(hidden_states).reshape(-1, H)
    router_logits = x @ jnp.asarray(router_w).T
    routing_weights = jax.nn.softmax(router_logits.astype(jnp.float32), axis=-1)
    top_k_weights, top_k_index = jax.lax.top_k(routing_weights, TOP_K)
    return np.asarray(top_k_index), np.asarray(top_k_weights, dtype=np.float32)


def kernel(hidden_states, router_w, gate_w, up_w, down_w):
    from concourse.bass_utils import run_bass_kernel_spmd

    hidden_states = np.asarray(hidden_states, dtype=np.float32)
    router_w = np.asarray(router_w, dtype=np.float32)
    gate_w = np.asarray(gate_w, dtype=np.float32)
    up_w = np.asarray(up_w, dtype=np.float32)
    down_w = np.asarray(down_w, dtype=np.float32)

    tki, tkw = _routing(hidden_states, router_w)
    xf = hidden_states.reshape(T, H)

    idx_list, w_list = [], []
    for e in range(E):
        sel = tki == e  # [T, 2]
        tok = sel.any(axis=1)
        idx = np.nonzero(tok)[0]
        w = np.where(sel[:, 0], tkw[:, 0], tkw[:, 1])[idx]
        idx_list.append(idx)
        w_list.append(w.astype(np.float32))

    max_ne = max(len(i) for i in idx_list)
    C = max(512, int(math.ceil(max_ne / 256.0)) * 256)
    NT128 = C // P

    nc = _build_program(C)

    in_maps = []
    for e in range(E):
        idx, w = idx_list[e], w_list[e]
        ne = len(idx)
        xg = np.zeros((C, H), np.float32)
        xg[:ne] = xf[idx]
        wp = np.zeros((C,), np.float32)
        wp[:ne] = w
        in_maps.append(
            {
                "x": np.ascontiguousarray(
                    xg.T.reshape(HC, P, C).transpose(1, 0, 2)
                ),
                "gw": np.ascontiguousarray(
                    gate_w[e].reshape(FB, P, HC, P).transpose(0, 3, 2, 1)
                ),
                "uw": np.ascontiguousarray(
                    up_w[e].reshape(FB, P, HC, P).transpose(0, 3, 2, 1)
                ),
                "dw": np.ascontiguousarray(
                    down_w[e].T.reshape(FB, P, H).transpose(1, 0, 2)
                ),
                "wt": np.ascontiguousarray(wp.reshape(NT128, P)),
            }
        )

    res = run_bass_kernel_spmd(nc, in_maps, core_ids=list(range(N_CORES)))

    out = np.zeros((T, H), np.float32)
    for e in range(E):
        idx = idx_list[e]
        y = res.results[e]["y"].reshape(C, H)
        out[idx] += y[: len(idx)]
    return out.reshape(B, S, H)



# revision 9
# speedup vs baseline: 1.4357x; 1.4357x over previous
"""Jamba sparse-MoE block on 8 Trainium2 NeuronCores (expert-parallel, fp8).

Strategy
--------
- Routing (router matmul + softmax + top-2) is computed with jax on the host
  using the exact op sequence of the reference so expert selection matches
  bit-for-bit. Tokens are dispatched per expert on the host; core e runs the
  dense gate/up/silu/mul/down FFN of expert e over its <=C assigned tokens.
- All matmuls run as fp8(e4m3) DoubleRow matmuls (256-deep contraction at
  0.5 PE cycles/row = 4x the fp32r MAC rate). Accuracy is recovered with a
  3-term hi/lo decomposition per logical matmul:
      W @ X ~= Whi@Xhi + Wfresh@Xlo + Wres@Xhi8
  where the lo terms carry the quantization residuals at power-of-2 scales
  chosen so every fp8 operand sits in e4m3's normal range and all terms of
  one matmul accumulate into a single PSUM at a matched scale (the down
  weight-residual term uses a second PSUM folded in with one DVE op).
  Per-matmul relative error ~1e-3; full-network error ~2e-3 (gate 2e-2).
- Weight-side splits are precomputed on the host. The device only quantizes
  hid: one fp8 cast (Act) plus one scalar_tensor_tensor (DVE) residual,
  which is self-correcting (lo computed from the actually-stored hi).
- Elementwise/PSUM tiles cover 1024 tokens (2 PSUM banks; matmuls write
  512-wide bank-sized regions) to amortize the ~650ns/instr engine dispatch.
- Phase A streams gate/up weights once (fb-outer, sync DMA queue), stages
  hid hi/lo to DRAM in fp8 (stores on the Act queue); x variants stream on
  the gpsimd queue ahead of the down-weight preload, which fills SBUF
  during phase A so phase B streams hid through a resident weight cache.
"""

import math
import numpy as np
import ml_dtypes
from contextlib import ExitStack

B, S, H, F, E, TOP_K = 4, 2048, 1024, 4096, 8, 2
T = B * S
N_CORES = 8
P = 128
HC = H // P  # 8 h-chunks (k side of gate/up)
FB = F // P  # 32 f-blocks

E4 = ml_dtypes.float8_e4m3


def _token_tiles(C):
    assert C % 128 == 0 and C >= 512
    tiles = [1024] * (C // 1024)
    if C % 1024:
        tiles.append(C % 1024)
    return tiles


def _halves(nt):
    out = []
    o = 0
    while o < nt:
        out.append((o, min(512, nt - o)))
        o += 512
    return out


_PROGRAM_CACHE = {}


def _build_program(C, H_=H, F_=F, act="Silu"):
    """SPMD program for one expert's FFN over C token slots (fp8 DoubleRow)."""
    key = (C, H_, F_, act)
    if key in _PROGRAM_CACHE:
        return _PROGRAM_CACHE[key]
    import concourse.bacc as bacc
    import concourse.mybir as mybir
    import concourse.tile as tile

    HC = H_ // P
    FB = F_ // P
    f32 = mybir.dt.float32
    fp8 = mybir.dt.float8e4
    AF = mybir.ActivationFunctionType
    PM = mybir.MatmulPerfMode
    ALU = mybir.AluOpType
    NT128 = C // P
    tiles = _token_tiles(C)
    bchunks = [256] * (C // 256) + ([128] if C % 256 else [])

    nc = bacc.Bacc("TRN2", target_bir_lowering=False, debug=False, num_devices=N_CORES)

    xh_d = nc.dram_tensor("xh", [P, HC, C], fp8, kind="ExternalInput")
    xl_d = nc.dram_tensor("xl", [P, HC, C], fp8, kind="ExternalInput")
    x8_d = nc.dram_tensor("x8", [P, HC, C], fp8, kind="ExternalInput")
    ga_d = nc.dram_tensor("ga", [FB, P, HC, P], fp8, kind="ExternalInput")
    gb_d = nc.dram_tensor("gb", [FB, P, HC, P], fp8, kind="ExternalInput")
    gc_d = nc.dram_tensor("gc", [FB, P, HC, P], fp8, kind="ExternalInput")
    ua_d = nc.dram_tensor("ua", [FB, P, HC, P], fp8, kind="ExternalInput")
    ub_d = nc.dram_tensor("ub", [FB, P, HC, P], fp8, kind="ExternalInput")
    uc_d = nc.dram_tensor("uc", [FB, P, HC, P], fp8, kind="ExternalInput")
    da_d = nc.dram_tensor("da", [P, FB, H_], fp8, kind="ExternalInput")
    db_d = nc.dram_tensor("db", [P, FB, H_], fp8, kind="ExternalInput")
    dc_d = nc.dram_tensor("dc", [P, FB, H_], fp8, kind="ExternalInput")
    wt_d = nc.dram_tensor("wt", [NT128, P], f32, kind="ExternalInput")
    y_d = nc.dram_tensor("y", [NT128, P, H_], f32, kind="ExternalOutput")
    hh_d = nc.dram_tensor("hh", [FB, P, C], fp8)  # internal staging
    hl_d = nc.dram_tensor("hl", [FB, P, C], fp8)

    def mm(ps, w_t, x_t, j, npairs, start_t, stop_t):
        nc.tensor.matmul(
            ps,
            w_t,
            x_t,
            start=(start_t and j == 0),
            stop=(stop_t and j == npairs - 1),
            perf_mode=PM.DoubleRow,
        )

    with tile.TileContext(nc) as tc:
        with ExitStack() as ctx:
            wtpool = ctx.enter_context(tc.tile_pool(name="wtp", bufs=1))
            dwpool = ctx.enter_context(tc.tile_pool(name="dwp", bufs=1))

            # ---- Phase A: hid = silu(g) * u, quantized hi/lo, staged ----
            with ExitStack() as actx:
                psa = actx.enter_context(tc.tile_pool(name="psa", bufs=2, space="PSUM"))
                xpool = actx.enter_context(tc.tile_pool(name="xp", bufs=1))
                wpool = actx.enter_context(tc.tile_pool(name="wp", bufs=12))
                sgpool = actx.enter_context(tc.tile_pool(name="sgp", bufs=2))
                hpool = actx.enter_context(tc.tile_pool(name="hp", bufs=2))
                h8pool = actx.enter_context(tc.tile_pool(name="h8p", bufs=2))

                xh_t = xpool.tile([P, HC, C], fp8)
                xl_t = xpool.tile([P, HC, C], fp8)
                x8_t = xpool.tile([P, HC, C], fp8)
                t0 = 0
                while t0 < C:
                    ntc = min(512, C - t0)
                    sl = (slice(None), slice(None), slice(t0, t0 + ntc))
                    nc.gpsimd.dma_start(xh_t[sl], xh_d.ap()[sl])
                    nc.gpsimd.dma_start(xl_t[sl], xl_d.ap()[sl])
                    nc.gpsimd.dma_start(x8_t[sl], x8_d.ap()[sl])
                    t0 += ntc
                # after x: routing weights; down-proj weights preload is
                # chunked per-fb inside the loop below so no single transfer
                # hogs the (serialized) DMA pipe and starves the fb stream
                wt_t = wtpool.tile([P, NT128], f32)
                nc.gpsimd.dma_start(wt_t[:], wt_d.ap().rearrange("n p -> p n"))
                da_t = dwpool.tile([P, FB, H_], fp8)
                db_t = dwpool.tile([P, FB, H_], fp8)
                dc_t = dwpool.tile([P, FB, H_], fp8)

                for fb in range(FB):
                    wts = []
                    for d in (ga_d, gb_d, gc_d, ua_d, ub_d, uc_d):
                        w_t = wpool.tile([P, HC, P], fp8)
                        nc.sync.dma_start(w_t[:], d.ap()[fb])
                        wts.append(w_t)
                    ga_t, gb_t, gc_t, ua_t, ub_t, uc_t = wts
                    for s_t, s_d in ((da_t, da_d), (db_t, db_d), (dc_t, dc_d)):
                        nc.gpsimd.dma_start(
                            s_t[:, fb, :], s_d.ap()[:, fb, :]
                        )
                    t0 = 0
                    for nt in tiles:
                        ps_g = psa.tile([P, 1024], f32, name="ps_g")
                        ps_u = psa.tile([P, 1024], f32, name="ps_u")
                        for ps in (ps_g, ps_u):
                            wabc = (ga_t, gb_t, gc_t) if ps is ps_g else (ua_t, ub_t, uc_t)
                            for ho, hn in _halves(nt):
                                ts = slice(t0 + ho, t0 + ho + hn)
                                for w_t, x_t, st, sp in (
                                    (wabc[0], xh_t, True, False),
                                    (wabc[1], xl_t, False, False),
                                    (wabc[2], x8_t, False, True),
                                ):
                                    for j in range(HC // 2):
                                        pr = slice(2 * j, 2 * j + 2)
                                        mm(ps[:, ho : ho + hn], w_t[:, pr, :],
                                           x_t[:, pr, ts], j, HC // 2, st, sp)
                        ts = slice(t0, t0 + nt)
                        sg = sgpool.tile([P, 1024], f32, name="sg")[:, :nt]
                        nc.scalar.activation(sg, ps_g[:, :nt], getattr(AF, act),
                                             scale=1.0 / 32.0)
                        hid32 = hpool.tile([P, 1024], f32, name="hid32")[:, :nt]
                        nc.vector.tensor_mul(hid32, sg, ps_u[:, :nt])
                        hh = h8pool.tile([P, 1024], fp8, name="hh")[:, :nt]
                        nc.scalar.activation(hh, hid32, AF.Copy, scale=1.0 / 32.0)
                        hl = h8pool.tile([P, 1024], fp8, name="hl")[:, :nt]
                        nc.vector.scalar_tensor_tensor(
                            hl, hh, -32.0, hid32, ALU.mult, ALU.add
                        )
                        nc.scalar.dma_start(hh_d.ap()[fb][:, ts], hh)
                        nc.scalar.dma_start(hl_d.ap()[fb][:, ts], hl)
                        t0 += nt

            # ---- Phase B: y[t,:] = wt[t]/128 * (128*hid @ dw.T) ----
            psb1 = ctx.enter_context(tc.tile_pool(name="psb1", bufs=4, space="PSUM"))
            htpool = ctx.enter_context(tc.tile_pool(name="htp", bufs=2))
            ypool = ctx.enter_context(tc.tile_pool(name="yp", bufs=2))

            NHB = H_ // 512  # psum-bank-sized slices of the output row
            t0 = 0
            for chunk in bchunks:
                hh_t = htpool.tile([P, FB, 256], fp8, name="hh_t")
                hl_t = htpool.tile([P, FB, 256], fp8, name="hl_t")
                nc.sync.dma_start(
                    hh_t[:, :, :chunk],
                    hh_d.ap()[:, :, t0 : t0 + chunk].rearrange("b f t -> f b t"),
                )
                nc.sync.dma_start(
                    hl_t[:, :, :chunk],
                    hl_d.ap()[:, :, t0 : t0 + chunk].rearrange("b f t -> f b t"),
                )
                for sub in range(chunk // P):
                    tt = t0 // P + sub
                    sv = slice(sub * P, (sub + 1) * P)
                    ps1 = psb1.tile([P, H_], f32, name="ps1")
                    for nh in range(NHB):
                        hv = slice(nh * 512, (nh + 1) * 512)
                        for j in range(FB // 2):
                            pr = slice(2 * j, 2 * j + 2)
                            mm(ps1[:, hv], hh_t[:, pr, sv], da_t[:, pr, hv],
                               j, FB // 2, True, False)
                        for j in range(FB // 2):
                            pr = slice(2 * j, 2 * j + 2)
                            mm(ps1[:, hv], hl_t[:, pr, sv], db_t[:, pr, hv],
                               j, FB // 2, False, False)
                        for j in range(FB // 2):
                            pr = slice(2 * j, 2 * j + 2)
                            mm(ps1[:, hv], hh_t[:, pr, sv], dc_t[:, pr, hv],
                               j, FB // 2, False, True)
                    y_sb = ypool.tile([P, H_], f32, name="y_sb")
                    nc.scalar.activation(
                        y_sb[:], ps1[:], AF.Copy, scale=wt_t[:, tt : tt + 1]
                    )
                    nc.scalar.dma_start(y_d.ap()[tt], y_sb[:])
                t0 += chunk
    nc.compile()
    _PROGRAM_CACHE[key] = nc
    return nc


def _routing(hidden_states, router_w):
    """Replicate the reference's routing ops exactly (same jax ops, default
    platform) so top-2 selection matches bit-for-bit."""
    import jax
    import jax.numpy as jnp

    x = jnp.asarray(hidden_states).reshape(-1, H)
    router_logits = x @ jnp.asarray(router_w).T
    routing_weights = jax.nn.softmax(router_logits.astype(jnp.float32), axis=-1)
    top_k_weights, top_k_index = jax.lax.top_k(routing_weights, TOP_K)
    return np.asarray(top_k_index), np.asarray(top_k_weights, dtype=np.float32)


def _split3(w, s_hi, s_fresh, s_res):
    """fp8 hi/fresh/residual triplet of w at the given power-of-2 scales."""
    wa = (s_hi * w).astype(E4)
    wb = (s_fresh * w).astype(E4)
    wc = (s_res * (w - wa.astype(np.float32) / s_hi)).astype(E4)
    return (
        np.ascontiguousarray(wa),
        np.ascontiguousarray(wb),
        np.ascontiguousarray(wc),
    )


def kernel(hidden_states, router_w, gate_w, up_w, down_w):
    from concourse.bass_utils import run_bass_kernel_spmd

    hidden_states = np.asarray(hidden_states, dtype=np.float32)
    router_w = np.asarray(router_w, dtype=np.float32)
    gate_w = np.asarray(gate_w, dtype=np.float32)
    up_w = np.asarray(up_w, dtype=np.float32)
    down_w = np.asarray(down_w, dtype=np.float32)

    tki, tkw = _routing(hidden_states, router_w)
    xf = hidden_states.reshape(T, H)

    idx_list, w_list = [], []
    for e in range(E):
        sel = tki == e  # [T, 2]
        tok = sel.any(axis=1)
        idx = np.nonzero(tok)[0]
        w = np.where(sel[:, 0], tkw[:, 0], tkw[:, 1])[idx]
        idx_list.append(idx)
        w_list.append(w.astype(np.float32))

    max_ne = max(len(i) for i in idx_list)
    C = max(512, int(math.ceil(max_ne / 128.0)) * 128)
    NT128 = C // P

    nc = _build_program(C)

    in_maps = []
    for e in range(E):
        idx, w = idx_list[e], w_list[e]
        ne = len(idx)
        xg = np.zeros((C, H), np.float32)
        xg[:ne] = xf[idx]
        wp = np.zeros((C,), np.float32)
        wp[:ne] = w / 128.0

        xt = np.ascontiguousarray(xg.T.reshape(HC, P, C).transpose(1, 0, 2))
        xh = xt.astype(E4)
        xl = (8.0 * (xt - xh.astype(np.float32))).astype(E4)
        x8 = (xt / 8.0).astype(E4)

        g = gate_w[e].reshape(FB, P, HC, P).transpose(0, 3, 2, 1).astype(np.float32)
        u = up_w[e].reshape(FB, P, HC, P).transpose(0, 3, 2, 1).astype(np.float32)
        d = down_w[e].T.reshape(FB, P, H).transpose(1, 0, 2).astype(np.float32)
        ga, gb, gc = _split3(g, 32.0, 4.0, 256.0)
        ua, ub, uc = _split3(u, 32.0, 4.0, 256.0)
        da, db, dc = _split3(d, 128.0, 4.0, 128.0)

        in_maps.append(
            {
                "xh": np.ascontiguousarray(xh),
                "xl": np.ascontiguousarray(xl),
                "x8": np.ascontiguousarray(x8),
                "ga": ga, "gb": gb, "gc": gc,
                "ua": ua, "ub": ub, "uc": uc,
                "da": da, "db": db, "dc": dc,
                "wt": np.ascontiguousarray(wp.reshape(NT128, P)),
            }
        )

    res = run_bass_kernel_spmd(nc, in_maps, core_ids=list(range(N_CORES)))

    out = np.zeros((T, H), np.float32)
    for e in range(E):
        idx = idx_list[e]
        y = np.asarray(res.results[e]["y"], dtype=np.float32).reshape(C, H)
        out[idx] += y[: len(idx)]
    return out.reshape(B, S, H)


# revision 13
# speedup vs baseline: 1.5558x; 1.0836x over previous
"""Jamba sparse-MoE block on 8 Trainium2 NeuronCores (expert-parallel, fp8).

Strategy
--------
- Routing (router matmul + softmax + top-2) is computed with jax on the host
  using the exact op sequence of the reference so expert selection matches
  bit-for-bit. Tokens are dispatched per expert on the host; core e runs the
  dense gate/up/silu/mul/down FFN of expert e over its <=C assigned tokens.
- All matmuls run as fp8(e4m3) DoubleRow matmuls (256-deep contraction at
  0.5 PE cycles/row = 4x the fp32r MAC rate). Accuracy is recovered with a
  3-term hi/lo decomposition per logical matmul:
      W @ X ~= Whi@Xhi + Wfresh@Xlo + Wres@Xhi
  where the lo terms carry the quantization residuals at power-of-2 scales
  chosen so the fp8 operands stay in (or near) e4m3's normal range and all
  three terms accumulate into a single PSUM at one matched scale.
  Per-matmul relative error ~1.3e-3; full-network error ~2e-3 (gate 2e-2).
- Weight-side splits are precomputed on the host. The device only quantizes
  hid: one fp8 cast (Act) plus one scalar_tensor_tensor (DVE) residual,
  which is self-correcting (lo computed from the actually-stored hi).
- Elementwise/PSUM tiles cover 1024 tokens (2 PSUM banks; matmuls write
  512-wide bank-sized regions) to amortize the ~650ns/instr engine dispatch.
- Phase A streams gate/up weights once (fb-outer, sync DMA queue), stages
  hid hi/lo to DRAM in fp8 (stores on the Act queue); x variants stream on
  the gpsimd queue ahead of the down-weight preload, which fills SBUF
  during phase A so phase B streams hid through a resident weight cache.
"""

import math
import numpy as np
import ml_dtypes
from contextlib import ExitStack

B, S, H, F, E, TOP_K = 4, 2048, 1024, 4096, 8, 2
T = B * S
N_CORES = 8
P = 128
HC = H // P  # 8 h-chunks (k side of gate/up)
FB = F // P  # 32 f-blocks

E4 = ml_dtypes.float8_e4m3


def _token_tiles(C):
    assert C % 128 == 0 and C >= 512
    tiles = [1024] * (C // 1024)
    if C % 1024:
        tiles.append(C % 1024)
    return tiles


def _halves(nt):
    out = []
    o = 0
    while o < nt:
        out.append((o, min(512, nt - o)))
        o += 512
    return out


_PROGRAM_CACHE = {}


def _build_program(C, H_=H, F_=F, act="Silu"):
    """SPMD program for one expert's FFN over C token slots (fp8 DoubleRow)."""
    key = (C, H_, F_, act)
    if key in _PROGRAM_CACHE:
        return _PROGRAM_CACHE[key]
    import concourse.bacc as bacc
    import concourse.mybir as mybir
    import concourse.tile as tile

    HC = H_ // P
    FB = F_ // P
    f32 = mybir.dt.float32
    fp8 = mybir.dt.float8e4
    AF = mybir.ActivationFunctionType
    PM = mybir.MatmulPerfMode
    ALU = mybir.AluOpType
    NT128 = C // P
    tiles = _token_tiles(C)
    rem = C - 128
    bchunks = [128] + [256] * (rem // 256) + ([128] if rem % 256 else [])

    nc = bacc.Bacc("TRN2", target_bir_lowering=False, debug=False, num_devices=N_CORES)

    xh_d = nc.dram_tensor("xh", [P, HC, C], fp8, kind="ExternalInput")
    xl_d = nc.dram_tensor("xl", [P, HC, C], fp8, kind="ExternalInput")
    ga_d = nc.dram_tensor("ga", [FB, P, HC, P], fp8, kind="ExternalInput")
    gb_d = nc.dram_tensor("gb", [FB, P, HC, P], fp8, kind="ExternalInput")
    gc_d = nc.dram_tensor("gc", [FB, P, HC, P], fp8, kind="ExternalInput")
    ua_d = nc.dram_tensor("ua", [FB, P, HC, P], fp8, kind="ExternalInput")
    ub_d = nc.dram_tensor("ub", [FB, P, HC, P], fp8, kind="ExternalInput")
    uc_d = nc.dram_tensor("uc", [FB, P, HC, P], fp8, kind="ExternalInput")
    da_d = nc.dram_tensor("da", [P, FB, H_], fp8, kind="ExternalInput")
    db_d = nc.dram_tensor("db", [P, FB, H_], fp8, kind="ExternalInput")
    dc_d = nc.dram_tensor("dc", [P, FB, H_], fp8, kind="ExternalInput")
    wt_d = nc.dram_tensor("wt", [NT128, P], f32, kind="ExternalInput")
    y_d = nc.dram_tensor("y", [NT128, P, H_], f32, kind="ExternalOutput")
    hh_d = nc.dram_tensor("hh", [FB, P, C], fp8)  # internal staging
    hl_d = nc.dram_tensor("hl", [FB, P, C], fp8)

    def mm(ps, w_t, x_t, j, npairs, start_t, stop_t):
        nc.tensor.matmul(
            ps,
            w_t,
            x_t,
            start=(start_t and j == 0),
            stop=(stop_t and j == npairs - 1),
            perf_mode=PM.DoubleRow,
        )

    with tile.TileContext(nc) as tc:
        with ExitStack() as ctx:
            wtpool = ctx.enter_context(tc.tile_pool(name="wtp", bufs=1))
            dwpool = ctx.enter_context(tc.tile_pool(name="dwp", bufs=1))

            # ---- Phase A: hid = silu(g) * u, quantized hi/lo, staged ----
            with ExitStack() as actx:
                psa = actx.enter_context(tc.tile_pool(name="psa", bufs=2, space="PSUM"))
                xpool = actx.enter_context(tc.tile_pool(name="xp", bufs=1))
                wpool = actx.enter_context(tc.tile_pool(name="wp", bufs=12))
                sgpool = actx.enter_context(tc.tile_pool(name="sgp", bufs=2))
                hpool = actx.enter_context(tc.tile_pool(name="hp", bufs=2))
                h8pool = actx.enter_context(tc.tile_pool(name="h8p", bufs=2))

                xh_t = xpool.tile([P, HC, C], fp8)
                xl_t = xpool.tile([P, HC, C], fp8)
                t0 = 0
                while t0 < C:
                    ntc = min(512, C - t0)
                    sl = (slice(None), slice(None), slice(t0, t0 + ntc))
                    nc.gpsimd.dma_start(xh_t[sl], xh_d.ap()[sl])
                    nc.gpsimd.dma_start(xl_t[sl], xl_d.ap()[sl])
                    t0 += ntc
                # after x: routing weights; down-proj weights preload is
                # chunked per-fb inside the loop below so no single transfer
                # hogs the (serialized) DMA pipe and starves the fb stream
                wt_t = wtpool.tile([P, NT128], f32)
                nc.gpsimd.dma_start(wt_t[:], wt_d.ap().rearrange("n p -> p n"))
                da_t = dwpool.tile([P, FB, H_], fp8)
                db_t = dwpool.tile([P, FB, H_], fp8)
                dc_t = dwpool.tile([P, FB, H_], fp8)

                for fb in range(FB):
                    wts = []
                    for d in (ga_d, gb_d, gc_d, ua_d, ub_d, uc_d):
                        w_t = wpool.tile([P, HC, P], fp8)
                        nc.sync.dma_start(w_t[:], d.ap()[fb])
                        wts.append(w_t)
                    ga_t, gb_t, gc_t, ua_t, ub_t, uc_t = wts
                    for s_t, s_d in ((da_t, da_d), (db_t, db_d), (dc_t, dc_d)):
                        nc.gpsimd.dma_start(
                            s_t[:, fb, :], s_d.ap()[:, fb, :]
                        )
                    t0 = 0
                    for nt in tiles:
                        ps_g = psa.tile([P, 1024], f32, name="ps_g")
                        ps_u = psa.tile([P, 1024], f32, name="ps_u")
                        for ps in (ps_g, ps_u):
                            wabc = (ga_t, gb_t, gc_t) if ps is ps_g else (ua_t, ub_t, uc_t)
                            for ho, hn in _halves(nt):
                                ts = slice(t0 + ho, t0 + ho + hn)
                                for w_t, x_t, st, sp in (
                                    (wabc[0], xh_t, True, False),
                                    (wabc[1], xl_t, False, False),
                                    (wabc[2], xh_t, False, True),
                                ):
                                    for j in range(HC // 2):
                                        pr = slice(2 * j, 2 * j + 2)
                                        mm(ps[:, ho : ho + hn], w_t[:, pr, :],
                                           x_t[:, pr, ts], j, HC // 2, st, sp)
                        ts = slice(t0, t0 + nt)
                        sg = sgpool.tile([P, 1024], f32, name="sg")[:, :nt]
                        nc.scalar.activation(sg, ps_g[:, :nt], getattr(AF, act),
                                             scale=1.0 / 32.0)
                        hid32 = hpool.tile([P, 1024], f32, name="hid32")[:, :nt]
                        nc.vector.tensor_mul(hid32, sg, ps_u[:, :nt])
                        hh = h8pool.tile([P, 1024], fp8, name="hh")[:, :nt]
                        nc.scalar.activation(hh, hid32, AF.Copy, scale=1.0 / 32.0)
                        hl = h8pool.tile([P, 1024], fp8, name="hl")[:, :nt]
                        nc.vector.scalar_tensor_tensor(
                            hl, hh, -32.0, hid32, ALU.mult, ALU.add
                        )
                        nc.scalar.dma_start(hh_d.ap()[fb][:, ts], hh)
                        nc.scalar.dma_start(hl_d.ap()[fb][:, ts], hl)
                        t0 += nt

            # ---- Phase B: y[t,:] = wt[t]/128 * (128*hid @ dw.T) ----
            psb1 = ctx.enter_context(tc.tile_pool(name="psb1", bufs=4, space="PSUM"))
            htpool = ctx.enter_context(tc.tile_pool(name="htp", bufs=2))
            ypool = ctx.enter_context(tc.tile_pool(name="yp", bufs=2))

            NHB = H_ // 512  # psum-bank-sized slices of the output row
            t0 = 0
            for chunk in bchunks:
                hh_t = htpool.tile([P, FB, 256], fp8, name="hh_t")
                hl_t = htpool.tile([P, FB, 256], fp8, name="hl_t")
                nc.sync.dma_start(
                    hh_t[:, :, :chunk],
                    hh_d.ap()[:, :, t0 : t0 + chunk].rearrange("b f t -> f b t"),
                )
                nc.sync.dma_start(
                    hl_t[:, :, :chunk],
                    hl_d.ap()[:, :, t0 : t0 + chunk].rearrange("b f t -> f b t"),
                )
                for sub in range(chunk // P):
                    tt = t0 // P + sub
                    sv = slice(sub * P, (sub + 1) * P)
                    ps1 = psb1.tile([P, H_], f32, name="ps1")
                    for nh in range(NHB):
                        hv = slice(nh * 512, (nh + 1) * 512)
                        for j in range(FB // 2):
                            pr = slice(2 * j, 2 * j + 2)
                            mm(ps1[:, hv], hh_t[:, pr, sv], da_t[:, pr, hv],
                               j, FB // 2, True, False)
                        for j in range(FB // 2):
                            pr = slice(2 * j, 2 * j + 2)
                            mm(ps1[:, hv], hl_t[:, pr, sv], db_t[:, pr, hv],
                               j, FB // 2, False, False)
                        for j in range(FB // 2):
                            pr = slice(2 * j, 2 * j + 2)
                            mm(ps1[:, hv], hh_t[:, pr, sv], dc_t[:, pr, hv],
                               j, FB // 2, False, True)
                    y_sb = ypool.tile([P, H_], f32, name="y_sb")
                    nc.scalar.activation(
                        y_sb[:], ps1[:], AF.Copy, scale=wt_t[:, tt : tt + 1]
                    )
                    nc.scalar.dma_start(y_d.ap()[tt], y_sb[:])
                t0 += chunk
    nc.compile()
    _PROGRAM_CACHE[key] = nc
    return nc


def _routing(hidden_states, router_w):
    """Replicate the reference's routing ops exactly (same jax ops, default
    platform) so top-2 selection matches bit-for-bit."""
    import jax
    import jax.numpy as jnp

    x = jnp.asarray(hidden_states).reshape(-1, H)
    router_logits = x @ jnp.asarray(router_w).T
    routing_weights = jax.nn.softmax(router_logits.astype(jnp.float32), axis=-1)
    top_k_weights, top_k_index = jax.lax.top_k(routing_weights, TOP_K)
    return np.asarray(top_k_index), np.asarray(top_k_weights, dtype=np.float32)


def _split3(w, s_hi, s_fresh, s_res):
    """fp8 hi/fresh/residual triplet of w at the given power-of-2 scales."""
    wa = (s_hi * w).astype(E4)
    wb = (s_fresh * w).astype(E4)
    wc = (s_res * (w - wa.astype(np.float32) / s_hi)).astype(E4)
    return (
        np.ascontiguousarray(wa),
        np.ascontiguousarray(wb),
        np.ascontiguousarray(wc),
    )


def kernel(hidden_states, router_w, gate_w, up_w, down_w):
    from concourse.bass_utils import run_bass_kernel_spmd

    hidden_states = np.asarray(hidden_states, dtype=np.float32)
    router_w = np.asarray(router_w, dtype=np.float32)
    gate_w = np.asarray(gate_w, dtype=np.float32)
    up_w = np.asarray(up_w, dtype=np.float32)
    down_w = np.asarray(down_w, dtype=np.float32)

    tki, tkw = _routing(hidden_states, router_w)
    xf = hidden_states.reshape(T, H)

    idx_list, w_list = [], []
    for e in range(E):
        sel = tki == e  # [T, 2]
        tok = sel.any(axis=1)
        idx = np.nonzero(tok)[0]
        w = np.where(sel[:, 0], tkw[:, 0], tkw[:, 1])[idx]
        idx_list.append(idx)
        w_list.append(w.astype(np.float32))

    max_ne = max(len(i) for i in idx_list)
    C = max(512, int(math.ceil(max_ne / 128.0)) * 128)
    NT128 = C // P

    nc = _build_program(C)

    in_maps = []
    for e in range(E):
        idx, w = idx_list[e], w_list[e]
        ne = len(idx)
        xg = np.zeros((C, H), np.float32)
        xg[:ne] = xf[idx]
        wp = np.zeros((C,), np.float32)
        wp[:ne] = w / 128.0

        xt = np.ascontiguousarray(xg.T.reshape(HC, P, C).transpose(1, 0, 2))
        xh = xt.astype(E4)
        xl = (8.0 * (xt - xh.astype(np.float32))).astype(E4)

        g = gate_w[e].reshape(FB, P, HC, P).transpose(0, 3, 2, 1).astype(np.float32)
        u = up_w[e].reshape(FB, P, HC, P).transpose(0, 3, 2, 1).astype(np.float32)
        d = down_w[e].T.reshape(FB, P, H).transpose(1, 0, 2).astype(np.float32)
        ga, gb, gc = _split3(g, 32.0, 4.0, 32.0)
        ua, ub, uc = _split3(u, 32.0, 4.0, 32.0)
        da, db, dc = _split3(d, 128.0, 4.0, 128.0)

        in_maps.append(
            {
                "xh": np.ascontiguousarray(xh),
                "xl": np.ascontiguousarray(xl),
                "ga": ga, "gb": gb, "gc": gc,
                "ua": ua, "ub": ub, "uc": uc,
                "da": da, "db": db, "dc": dc,
                "wt": np.ascontiguousarray(wp.reshape(NT128, P)),
            }
        )

    res = run_bass_kernel_spmd(nc, in_maps, core_ids=list(range(N_CORES)))

    out = np.zeros((T, H), np.float32)
    for e in range(E):
        idx = idx_list[e]
        y = np.asarray(res.results[e]["y"], dtype=np.float32).reshape(C, H)
        out[idx] += y[: len(idx)]
    return out.reshape(B, S, H)
